# revision 1
# baseline (speedup 1.0000x reference)
"""Multi-head self-attention (B=2, T=2048, C=1024, H=16) on 8 trn2 cores.

Sharding: core c -> batch b = c//4, heads 4*(c%4) .. 4*(c%4)+3.
Each core: QKV projection for its 4 heads, causal attention in S^T layout
(keys on partitions), partial output projection over its heads' rows of Wo.
Host sums the 4 partials per batch element and adds bo.

All matmuls run in float32r (fp32 with 12-bit mantissa, full PE rate).
"""
import sys

sys.path.insert(0, "/opt/trn_rl_repo")

import numpy as np

B, T, C, H = 2, 2048, 1024, 16
HD = C // H            # 64
NCORES = 8
HPC = H // (NCORES // B)   # heads per core = 4
QB = 128               # q block (columns of S^T)
KB = 128               # k chunk (partitions of S^T)
NJ = T // KB           # 16
NI = T // QB           # 16
SLAB = 512             # q columns processed per attention pass
NSLAB = T // SLAB      # 4
BPS = SLAB // QB       # q blocks per slab = 4
CI = C // 128          # 8 contraction chunks for projections
SCALE = HD ** -0.5

_cache = {}


def _round_fp32r(x: np.ndarray) -> np.ndarray:
    u = np.ascontiguousarray(x, dtype=np.float32).view(np.uint32)
    r = (u + 0x7FF + ((u >> 12) & 1)) & np.uint32(0xFFFFF000)
    return r.view(np.float32)


def _build_plan(mask_bool: np.ndarray):
    """mask_bool: [T, T] (q, k). Returns per (j, i) block types and tiles.

    type 0 = all valid (no mask work), 1 = all masked (skip), 2 = mixed.
    Tiles are stored transposed to match S^T ([k_local, q_local])."""
    btype = np.zeros((NJ, NI), dtype=np.int32)
    tidx = np.full((NJ, NI), -1, dtype=np.int32)
    tiles = []
    tile_map = {}
    for j in range(NJ):
        for i in range(NI):
            sub = mask_bool[i * QB:(i + 1) * QB, j * KB:(j + 1) * KB]
            if sub.all():
                btype[j, i] = 0
            elif not sub.any():
                btype[j, i] = 1
            else:
                btype[j, i] = 2
                key = sub.tobytes()
                if key not in tile_map:
                    tile_map[key] = len(tiles)
                    tiles.append(sub.T.astype(np.float32))
                tidx[j, i] = tile_map[key]
    if not tiles:
        tiles.append(np.ones((KB, QB), dtype=np.float32))
    return btype, tidx, np.stack(tiles)


def _build_program(btype, tidx, n_tiles, apply_qk_bias, apply_v_bias):
    import concourse.bass as bass
    import concourse.tile as tile
    import concourse.mybir as mybir
    from concourse import bacc

    F32 = mybir.dt.float32
    F32R = mybir.dt.float32r
    AF = mybir.ActivationFunctionType
    MULT = mybir.AluOpType.mult

    nc = bacc.Bacc("TRN2", target_bir_lowering=False, debug=False)
    xt_d = nc.dram_tensor("xt", [C, T], F32R, kind="ExternalInput").ap()
    wqk_d = nc.dram_tensor("wqk", [C, 4 * 128], F32R, kind="ExternalInput").ap()
    wv_d = nc.dram_tensor("wv", [C, HPC * HD], F32R, kind="ExternalInput").ap()
    wo_d = nc.dram_tensor("wo", [HPC * HD, C], F32R, kind="ExternalInput").ap()
    mask_d = nc.dram_tensor("masks", [n_tiles, KB, QB], F32,
                            kind="ExternalInput").ap()
    bqk_d = nc.dram_tensor("bqk", [128, 4], F32, kind="ExternalInput").ap()
    bv_d = nc.dram_tensor("bv", [128, 2], F32, kind="ExternalInput").ap()
    zero_d = nc.dram_tensor("zeros", [64, T], F32R, kind="ExternalInput").ap()
    out_d = nc.dram_tensor("out", [T, C], F32, kind="ExternalOutput").ap()

    with tile.TileContext(nc) as tc:
        with tc.tile_pool(name="weights", bufs=1) as wpool, \
             tc.tile_pool(name="acts", bufs=1) as apool:
            # ---- resident SBUF tensors ----
            wo = wpool.tile([128, 2, C], F32R)            # head-pair chunks
            masks = wpool.tile([128, n_tiles * QB], F32)
            bqk = wpool.tile([128, 4], F32)
            bv = wpool.tile([128, 2], F32)
            # q tiles hold (q_hA | q_hB) on partitions 0-63 / 64-127.
            # k is stored zero-padded per head (other head's partitions are
            # zero) so S matmuls present K=128 to the PE — K=64 matmuls do
            # not register as HAM activity and leave the clock at 1.2 GHz.
            qp = [apool.tile([128, T], F32R, tag=f"qp{i}", name=f"qp{i}")
                  for i in range(2)]
            kz = [apool.tile([128, T], F32R, tag=f"kz{i}", name=f"kz{i}")
                  for i in range(4)]          # index = 2*pair + head
            vaug = apool.tile([128, NJ, HPC * (HD + 1)], F32R)


            # ---- QKV projection ----
            # q/k in transposed layout: psum[c_out_pair, t] = W^T x^T
            with tc.tile_pool(name="xtp", bufs=1) as xtp, \
                 tc.tile_pool(name="pproj", bufs=4, space="PSUM") as pp:
                xt = xtp.tile([128, CI, T], F32R)      # x^T, c_in chunked
                wqk = xtp.tile([128, CI, 512], F32R)
                wv = xtp.tile([128, CI, HPC * HD], F32R)
                for ci in range(CI):
                    qt4 = T // 4
                    for qn in range(4):
                        nc.sync.dma_start(
                            xt[:, ci, qn * qt4:(qn + 1) * qt4],
                            xt_d[ci * 128:(ci + 1) * 128,
                                 qn * qt4:(qn + 1) * qt4])
                    nc.sync.dma_start(wqk[:, ci, :],
                                      wqk_d[ci * 128:(ci + 1) * 128, :])
                    nc.sync.dma_start(wv[:, ci, :],
                                      wv_d[ci * 128:(ci + 1) * 128, :])
                # lower-priority loads and inits (after the projection inputs)
                nc.sync.dma_start(wo[:, 0, :], wo_d[0:128, :])
                nc.sync.dma_start(wo[:, 1, :], wo_d[128:256, :])
                for t in range(n_tiles):
                    nc.sync.dma_start(masks[:, t * QB:(t + 1) * QB], mask_d[t])
                nc.sync.dma_start(bqk[:], bqk_d)
                nc.sync.dma_start(bv[:], bv_d)
                va = vaug[:].rearrange("p j (h d) -> p j h d", h=HPC)
                nc.vector.tensor_copy(
                    va[:, :, :, HD:HD + 1],
                    nc.const_aps.tensor(1.0, (128, NJ, HPC, 1)))
                for p in range(2):
                    nc.sync.dma_start(kz[2 * p][64:128, :], zero_d)
                    nc.sync.dma_start(kz[2 * p + 1][0:64, :], zero_d)
                for co in (0, 1, "v", 2, 3):
                    if co == "v":
                        # v projection between the two pairs' qk so pair 0's
                        # attention (needs qk 0/1 + V) can start at full
                        # stride while pair 1's qk still projects
                        for tj in range(NJ):
                            psv = pp.tile([128, HPC * HD], F32, tag="pv",
                                          name="psv")
                            for ci in range(CI):
                                nc.tensor.matmul(
                                    psv[:],
                                    xt[:, ci, tj * 128:(tj + 1) * 128],
                                    wv[:, ci, :],
                                    start=(ci == 0), stop=(ci == CI - 1))
                            nc.vector.tensor_copy(
                                va[:, tj, :, 0:HD],
                                psv[:].rearrange("p (h d) -> p h d", h=HPC))
                        continue
                    pair, is_k = co // 2, co % 2
                    for ts in range(T // 512):
                        sl = slice(ts * 512, (ts + 1) * 512)
                        ps = pp.tile([128, 512], F32, tag="pqk")
                        for ci in range(CI):
                            nc.tensor.matmul(
                                ps[:],
                                wqk[:, ci, co * 128:(co + 1) * 128],
                                xt[:, ci, sl],
                                start=(ci == 0), stop=(ci == CI - 1))
                        if is_k:
                            dsts = [(kz[2 * pair][0:64, sl], ps[0:64, :],
                                     bqk[0:64, co:co + 1]),
                                    (kz[2 * pair + 1][64:128, sl],
                                     ps[64:128, :], bqk[64:128, co:co + 1])]
                        else:
                            dsts = [(qp[pair][:, sl], ps[:],
                                     bqk[:, co:co + 1])]
                        for dst_ap, src_ap, b_ap in dsts:
                            if apply_qk_bias:
                                nc.scalar.activation(dst_ap, src_ap,
                                                     AF.Identity, bias=b_ap,
                                                     scale=1.0)
                            else:
                                nc.vector.tensor_copy(dst_ap, src_ap)

            # ---- attention (S^T layout) ----
            # Chunk pairs share one 2-bank PSUM tile per head so exp covers
            # both in a single ACTIVATE. PV accumulators are per (pair, head)
            # so one pair's softmax division overlaps the other pair's
            # matmuls, keeping the PE continuously busy (HAM stays warm).
            with tc.tile_pool(name="attnp", bufs=1) as attnp:
              attn = [attnp.tile([128, T], F32R, tag=f"attn{p}",
                                 name=f"attn{p}") for p in range(2)]
              with tc.tile_pool(name="psattn", bufs=1, space="PSUM") as sp, \
                   tc.tile_pool(name="psout", bufs=1, space="PSUM") as op, \
                   tc.tile_pool(name="ptp", bufs=5) as ptp, \
                   tc.tile_pool(name="divp", bufs=2) as divp:
                  sums_all = [divp.tile([1, T], F32, tag=f"sums{r}",
                                        name=f"sums{r}", bufs=1)
                              for r in range(4)]
                  pending = []
                  for pair in range(2):
                      q_t = qp[pair]
                      for s in range(NSLAB):
                          if pair == 1 and s >= 1 and pending:
                              pr, hh_ = pending.pop(0)
                              _division_hl(pr, hh_)
                          i_lo, i_hi = s * BPS, (s + 1) * BPS
                          # chunk runs for this slab (shared by all heads)
                          chunks = []
                          for j in range(NJ):
                              live = [i for i in range(i_lo, i_hi)
                                      if btype[j, i] != 1]
                              if live:
                                  chunks.append((j, min(live), max(live)))
                          out_ps = [op.tile([HD + 1, SLAB], F32,
                                            tag=f"outps{_hl}",
                                            name=f"outps{_hl}", bufs=2)
                                    for _hl in range(2)]
                          written = np.zeros(BPS, dtype=bool)
                          for cn, (j, i0, i1) in enumerate(chunks):
                              n_cols = (i1 - i0 + 1) * QB
                              r0 = i0 - i_lo
                              # S^T for both heads into the two banks of one
                              # psum tile; one exp and one mask op cover both
                              sps = sp.tile([128, 2, SLAB], F32,
                                            tag="sst", name="sst", bufs=2)
                              for hl in range(2):
                                  nc.tensor.matmul(
                                      sps[:, hl, 0:n_cols],
                                      kz[2 * pair + hl][:, j * KB:(j + 1) * KB],
                                      q_t[:, i0 * QB:i0 * QB + n_cols],
                                      start=True, stop=True)
                              pt = ptp.tile([128, 2, SLAB], F32R, tag="pt",
                                            name="pt")
                              nc.scalar.activation(pt[:, :, 0:n_cols],
                                                   sps[:, :, 0:n_cols],
                                                   AF.Exp, scale=SCALE)
                              for i in range(i0, i1 + 1):
                                  rel = (i - i0) * QB
                                  if btype[j, i] == 2:
                                      ti = tidx[j, i]
                                      m2 = masks[:, ti * QB:(ti + 1) * QB] \
                                          .unsqueeze(1).broadcast_to(
                                              [128, 2, QB])
                                      nc.gpsimd.tensor_tensor(
                                          out=pt[:, :, rel:rel + QB],
                                          in0=pt[:, :, rel:rel + QB],
                                          in1=m2, op=MULT)
                                  elif btype[j, i] == 1:
                                      nc.gpsimd.memset(pt[:, :, rel:rel + QB],
                                                       0.0)
                              # PV accumulation (runs are <= 512 so no bank
                              # crossing; split only on first-write transitions)
                              segs = []
                              c = r0 * QB
                              end = (i1 - i_lo + 1) * QB
                              while c < end:
                                  st = written[c // QB]
                                  cc = c + QB
                                  while cc < end and written[cc // QB] == st:
                                      cc += QB
                                  segs.append((c, cc, not st))
                                  c = cc
                              last = cn == len(chunks) - 1
                              for hl in range(2):
                                  hh = 2 * pair + hl
                                  for (c0, c1, st_flag) in segs:
                                      nc.tensor.matmul(
                                          out_ps[hl][:, c0:c1],
                                          vaug[:, j, hh * (HD + 1):
                                               (hh + 1) * (HD + 1)],
                                          pt[:, hl, c0 - r0 * QB:c1 - r0 * QB],
                                          start=st_flag, stop=last,
                                          skip_group_check=True)
                              for rr in range(r0, i1 - i_lo + 1):
                                  written[rr] = True
                          # stash undivided PV output + denominators; the
                          # division happens once per pair (keeps the slab
                          # boundary free of cross-engine chains)
                          for hl in range(2):
                              row = 2 * pair + hl
                              dst = attn[pair][64 * hl:64 * hl + 64,
                                               s * SLAB:(s + 1) * SLAB]
                              nc.vector.tensor_copy(dst, out_ps[hl][0:HD, :])
                              nc.vector.tensor_copy(
                                  sums_all[row][0:1,
                                                s * SLAB:(s + 1) * SLAB],
                                  out_ps[hl][HD:HD + 1, :])
                      # consolidated softmax division for this pair —
                      # pair 0's is deferred into pair 1's emission so its
                      # gpsimd broadcast never blocks pair 1's mask ops
                      def _division_hl(pair, hl):
                          row = 2 * pair + hl
                          rec1 = divp.tile([1, T], F32, tag="rec1",
                                           name="rec1")
                          rec128 = divp.tile([128, T], F32, tag="rec128",
                                             name="rec128")
                          nc.vector.reciprocal_approx_fast(
                              rec1[:], sums_all[row][:])
                          nc.gpsimd.partition_broadcast(rec128[:], rec1[:])
                          dst = attn[pair][64 * hl:64 * hl + 64, :]
                          nc.vector.tensor_tensor(
                              out=dst, in0=dst,
                              in1=rec128[64 * hl:64 * hl + 64, :], op=MULT)
                          if apply_v_bias:
                              nc.vector.tensor_scalar(
                                  out=dst, in0=dst,
                                  scalar1=bv[64 * hl:64 * hl + 64,
                                             pair:pair + 1],
                                  scalar2=None, op0=mybir.AluOpType.add)
                      for _hl in range(2):
                          pending.append((pair, _hl))
                      if pair == 1:
                          for pr, hh_ in pending:
                              _division_hl(pr, hh_)
                          pending = []

              # ---- output projection (partial; host sums over cores) ----
              with tc.tile_pool(name="psop", bufs=3, space="PSUM") as opp, \
                   tc.tile_pool(name="osb2", bufs=3) as osb2:
                  for ts in range(NI):
                      ps = opp.tile([128, C], F32, tag="opps", name="opps")
                      for pair in range(2):
                          for n0 in range(0, C, 512):
                              nc.tensor.matmul(
                                  ps[:, n0:n0 + 512],
                                  attn[pair][:, ts * 128:(ts + 1) * 128],
                                  wo[:, pair, n0:n0 + 512],
                                  start=(pair == 0), stop=(pair == 1))
                      ot = osb2.tile([128, C], F32, tag="ot", name="ot")
                      nc.vector.tensor_copy(ot[:], ps[:])
                      nc.sync.dma_start(out_d[ts * 128:(ts + 1) * 128, :],
                                        ot[:])

    nc.compile()
    return nc


def _get_program(mask_bool, apply_qk_bias, apply_v_bias):
    key = (mask_bool.tobytes(), apply_qk_bias, apply_v_bias)
    if key not in _cache:
        btype, tidx, tiles = _build_plan(mask_bool)
        nc = _build_program(btype, tidx, len(tiles), apply_qk_bias,
                            apply_v_bias)
        _cache[key] = (nc, tiles)
    return _cache[key]


def kernel(x, attention_mask, Wqkv, bqkv, Wo, bo, _trace=False):
    from concourse.bass_utils import run_bass_kernel_spmd

    x = np.asarray(x, dtype=np.float32)
    mask_bool = np.asarray(attention_mask)[0, 0] != 0
    Wqkv = np.asarray(Wqkv, dtype=np.float32)
    bqkv = np.asarray(bqkv, dtype=np.float32)
    Wo = np.asarray(Wo, dtype=np.float32)
    bo = np.asarray(bo, dtype=np.float32)

    apply_qk_bias = bool(np.any(bqkv[:2 * C]))
    apply_v_bias = bool(np.any(bqkv[2 * C:]))
    nc, tiles = _get_program(mask_bool, apply_qk_bias, apply_v_bias)

    xts = [_round_fp32r(x[b].T) for b in range(B)]
    zeros = np.zeros((64, T), dtype=np.float32)
    in_maps = []
    for c in range(NCORES):
        b, g = divmod(c, NCORES // B)
        hs = [HPC * g + i for i in range(HPC)]
        # wqk column chunks: [q_h0|q_h1, k_h0|k_h1, q_h2|q_h3, k_h2|k_h3]
        cols, bias_cols = [], []
        for pair in range(2):
            ha, hb = hs[2 * pair], hs[2 * pair + 1]
            for base in (0, C):  # q then k offset in Wqkv columns
                cols.append(Wqkv[:, base + ha * HD:base + (ha + 1) * HD])
                cols.append(Wqkv[:, base + hb * HD:base + (hb + 1) * HD])
                bias_cols.append(np.concatenate([
                    bqkv[base + ha * HD:base + (ha + 1) * HD],
                    bqkv[base + hb * HD:base + (hb + 1) * HD]]))
        wqk_c = _round_fp32r(np.concatenate(cols, axis=1))
        bqk_c = np.stack(bias_cols, axis=1).astype(np.float32)
        wv_c = _round_fp32r(np.concatenate(
            [Wqkv[:, 2 * C + h * HD:2 * C + (h + 1) * HD] for h in hs], axis=1))
        wo_c = _round_fp32r(np.concatenate(
            [Wo[h * HD:(h + 1) * HD, :] for h in hs], axis=0))
        bv_c = np.zeros((128, 2), dtype=np.float32)
        for pair in range(2):
            ha, hb = hs[2 * pair], hs[2 * pair + 1]
            bv_c[0:HD, pair] = bqkv[2 * C + ha * HD:2 * C + (ha + 1) * HD]
            bv_c[HD:128, pair] = bqkv[2 * C + hb * HD:2 * C + (hb + 1) * HD]
        in_maps.append({
            "xt": xts[b], "wqk": wqk_c, "wv": wv_c, "wo": wo_c,
            "masks": tiles, "bqk": bqk_c, "bv": bv_c, "zeros": zeros,
        })

    kwargs = {}
    if _trace:
        kwargs = dict(trace=True, trace_cores=[0])
    res = run_bass_kernel_spmd(nc, in_maps, core_ids=list(range(NCORES)),
                               **kwargs)
    out = np.empty((B, T, C), dtype=np.float32)
    gpb = NCORES // B
    for b in range(B):
        acc = res.results[b * gpb]["out"].astype(np.float32)
        for g in range(1, gpb):
            acc = acc + res.results[b * gpb + g]["out"]
        out[b] = acc + bo
    if _trace:
        kernel._last_results = res
    return out

